# revision 4
# baseline (speedup 1.0000x reference)
"""Trainium2 Bass kernel v2 for nn_Mlp_13099650253522 (BitNet-ternary MLP).

  h = gelu(x @ ter_quant(w1).T + b1);  y = h @ ter_quant(w2).T + b2
  ter_quant(w) = clip(round(w / g), -1, 1) * g,  g = mean(|w|) + 1e-5

Design (8 cores, data-parallel over 12608 tokens, 1576/core):
 - PE paces at N/2.4GHz+2.5ns per matmul when never stalled; everything
   is scheduled so the PE stream (1152 matmuls of N=394) never waits:
   * weights quantized to fp8 {-1,0,+1} (sign-flipped: t = (w<=-g/2)-(w>=g/2))
     slab-by-slab, racing ahead of fc1 consumption
   * psum: 2 big tiles [128,4,400] (4 token-tiles each); one ACT gelu per
     hc over all 4 token tiles (amortizes ACT fixed cost 4x)
   * t-inner matmul order reuses each LDWEIGHTS 4x
 - w1 [128, (hslab kd i)] fp32 streams first (12 chunks + DVE |.| reduces),
   gamma1 via gpsimd partition_all_reduce; 2-pass DVE quant
   (tensor_scalar is_ge; scalar_tensor_tensor is_le - prev).
 - w2 same layout [128, (dc kh i)]: pass1 streams for gamma2 (ACT Abs with
   accum_out row-sums, keeps DVE free), pass2 re-streams for quant.
 - h and y epilogues batched over 1576 tokens; y in bf16 (host upcasts).
 - w1 fp32 staging bufs are recycled as h tiles (same pool tag ring).
"""
import sys

for _p in ("/root/.axon_site", "/root/.axon_site/_ro/trn_rl_repo",
           "/root/.axon_site/_ro/pypackages", "/opt/trn_rl_repo"):
    if _p not in sys.path:
        sys.path.append(_p)

import ml_dtypes
import numpy as np

from concourse import bacc
import concourse.mybir as mybir
from concourse import bass_isa
from concourse.tile import TileContext
from concourse.tile_rust import add_dep_helper
from concourse.bass_utils import run_bass_kernel_spmd

FP32 = mybir.dt.float32
BF16 = mybir.dt.bfloat16
FP8 = mybir.dt.float8e4
Act = mybir.ActivationFunctionType
Alu = mybir.AluOpType
AxX = mybir.AxisListType.X

GELU_FUNC = None  # resolved to Act.Gelu at build; overridable for CoreSim

N_CORES = 8
B, S, D, H = 64, 197, 768, 3072
TOK = B * S                 # 12608
TOK_PER = TOK // N_CORES    # 1576
NT = 4
TN = TOK_PER // NT          # 394
KD = D // 128               # 6
KH = H // 128               # 24
W1C = 12                    # w1 dma chunks [128, 1536] (2 h-slabs each)
W2C = 12                    # w2 dma chunks [128, 1536] (12 kh-blocks each)
EPS = 1e-5
WEL = D * H                 # elements per weight matrix


def build():
    act_fn = GELU_FUNC if GELU_FUNC is not None else Act.Gelu
    nc = bacc.Bacc("TRN2", target_bir_lowering=False, debug=False)
    xt = nc.declare_dram_parameter("xt", [D, TOK_PER], BF16, isOutput=False)
    wt1 = nc.declare_dram_parameter("wt1", [128, KH * 768], FP32, isOutput=False)
    wt2 = nc.declare_dram_parameter("wt2", [128, KD * 3072], FP32, isOutput=False)
    b1r = nc.declare_dram_parameter("b1r", [128, KH], FP32, isOutput=False)
    b2r = nc.declare_dram_parameter("b2r", [128, KD], FP32, isOutput=False)
    yt = nc.declare_dram_parameter("yt", [D, TOK_PER], BF16, isOutput=True)

    with TileContext(nc) as tc:
        with (
            tc.tile_pool(name="singles", bufs=1) as singles,
            tc.tile_pool(name="wstage", bufs=W1C + 1) as wstage,  # w1 -> h
            tc.tile_pool(name="wf2", bufs=4) as wf2p,         # w2 fp32 stream
            tc.tile_pool(name="t1p", bufs=KH) as t1p,         # w1q slabs
            tc.tile_pool(name="t2p", bufs=KH) as t2p,         # w2q pieces
            tc.tile_pool(name="xbp", bufs=KD) as xbp,         # x tiles
            tc.tile_pool(name="absp", bufs=2) as absp,        # ACT |w2| scratch
            tc.tile_pool(name="apool", bufs=1) as apool,      # quant pass1
            # bufs=1 serializes the DVE quant stream in emission order so
            # the first slabs' semaphores land early (no scheduler reorder)
            tc.tile_pool(name="ybp", bufs=2) as ybp,          # y staging
            tc.tile_pool(name="psp", bufs=2, space="PSUM") as psp,
        ):
            # --- w1 DMA issues first: 3 queues (sync/scalar/gpsimd).
            # gpsimd DMAs go before the gpsimd warm-up ops so they are not
            # queued behind the ~10us custom-op library load. ---
            acc1 = singles.tile([128, W1C + 1], FP32, tag="acc1")
            wc = []
            whalf = []
            _w1engs = (nc.sync, nc.scalar)
            w1_dmas = []
            for c in range(W1C - 1):
                w = wstage.tile([128, 1536], FP32, tag="wc", name=f"w1c{c}")
                wc.append(w)
            for i in (0, 1):
                w = wstage.tile([128, 768], FP32, tag="wc", name=f"w1h{i}")
                whalf.append(w)
            # w1 chunks 0..7 first, then x (so all x lands before fc1
            # starts), then the last w1 chunks (they gate gamma anyway)
            for c in range(8):
                dma = _w1engs[c % 2].dma_start(
                    out=wc[c], in_=wt1[:, c * 1536:(c + 1) * 1536])
                w1_dmas.append(dma)
            xb6 = []
            for kd in range(KD):
                xbt = xbp.tile([128, TOK_PER], BF16, tag="xb")
                eng = nc.sync if kd % 2 == 0 else nc.scalar
                eng.dma_start(out=xbt, in_=xt[kd * 128:(kd + 1) * 128, :])
                xb6.append(xbt)
            for c in range(8, W1C - 1):
                dma = _w1engs[c % 2].dma_start(
                    out=wc[c], in_=wt1[:, c * 1536:(c + 1) * 1536])
                w1_dmas.append(dma)
            wh_dmas = []
            for i in (0, 1):
                base = (W1C - 1) * 1536 + i * 768
                dma = _w1engs[i].dma_start(out=whalf[i],
                                           in_=wt1[:, base:base + 768])
                wh_dmas.append(dma)

            # --- engine warm-up: gpsimd library + ACT tables ---
            dmy = singles.tile([128, 1], FP32, tag="dmy")
            nc.gpsimd.memset(dmy, 0.0)
            dmy2 = singles.tile([128, 1], FP32, tag="dmy2")
            nc.gpsimd.partition_all_reduce(dmy2, dmy, channels=128,
                                           reduce_op=bass_isa.ReduceOp.add)
            dmyg = singles.tile([128, 1], FP32, tag="dmyg")
            nc.scalar.activation(dmyg, dmy, act_fn)
            dmya = singles.tile([128, 1], FP32, tag="dmya")
            nc.scalar.activation(dmya, dmy, Act.Abs)

            # --- w1 |.| row-sum reduces: evens on DVE, odds on ACT ---
            for c in range(W1C - 1):
                if c % 2 == 0:
                    nc.vector.tensor_reduce(out=acc1[:, c:c + 1], in_=wc[c],
                                            axis=AxX, op=Alu.add,
                                            apply_absolute_value=True)
                else:
                    absw = absp.tile([128, 1536], BF16, tag="abs",
                                     name=f"absw{c}")
                    nc.scalar.activation(absw, wc[c], Act.Abs,
                                         accum_out=acc1[:, c:c + 1])
            nc.vector.tensor_reduce(out=acc1[:, W1C - 1:W1C], in_=whalf[0],
                                    axis=AxX, op=Alu.add,
                                    apply_absolute_value=True)
            abswh = absp.tile([128, 768], BF16, tag="abs")
            nc.scalar.activation(abswh, whalf[1], Act.Abs,
                                 accum_out=acc1[:, W1C:W1C + 1])

            b1sb = singles.tile([128, KH], FP32, tag="b1sb")
            nc.sync.dma_start(out=b1sb, in_=b1r[:, :])
            b2sb = singles.tile([128, KD], FP32, tag="b2sb")
            nc.sync.dma_start(out=b2sb, in_=b2r[:, :])

            def gamma_chain(acc, n_cols, tag):
                """|w| sums [128,n] -> (thr=+g/2, -g/2, -g) bcast [128,1]."""
                rs = singles.tile([128, 1], FP32, tag=tag + "_rs")
                nc.vector.tensor_reduce(out=rs, in_=acc[:, 0:n_cols],
                                        axis=AxX, op=Alu.add)
                ar = singles.tile([128, 1], FP32, tag=tag + "_ar")
                nc.gpsimd.partition_all_reduce(ar, rs, channels=128,
                                               reduce_op=bass_isa.ReduceOp.add)
                # thrp/thrn/gm all direct from ar (parallel, not chained)
                thrp = singles.tile([128, 1], FP32, tag=tag + "_tp")
                nc.vector.tensor_scalar(
                    out=thrp, in0=ar, scalar1=0.5 / WEL,
                    scalar2=EPS * 0.5, op0=Alu.mult, op1=Alu.add)
                thrn = singles.tile([128, 1], FP32, tag=tag + "_tn")
                nc.vector.tensor_scalar(
                    out=thrn, in0=ar, scalar1=-0.5 / WEL,
                    scalar2=-EPS * 0.5, op0=Alu.mult, op1=Alu.add)
                gm = singles.tile([128, 1], FP32, tag=tag + "_gm")
                nc.vector.tensor_scalar(
                    out=gm, in0=ar, scalar1=-1.0 / WEL,
                    scalar2=-EPS, op0=Alu.mult, op1=Alu.add)
                return thrp, thrn, gm

            # --- PE clock warm-up: dummy matmuls gated on late w1 chunks so
            # the HAM ramp completes right before the first real group ---
            dmw = singles.tile([128, 522], BF16, tag="dmw")
            nc.gpsimd.memset(dmw, 0.0)
            psd = psp.tile([128, NT, 512], FP32, tag="ps", name="psdummy")
            _gates = [(w1_dmas[6], 10), (w1_dmas[8], 10), (w1_dmas[9], 12),
                      (w1_dmas[10], 12), (wh_dmas[0], 8), (wh_dmas[1], 8)]
            for gate_dma, nmm in _gates:
                for i in range(nmm):
                    mm = nc.tensor.matmul(psd[:, 0, 0:TN], dmw[:, 0:128],
                                          dmw[:, 128:128 + TN],
                                          start=True, stop=True)
                    if i == 0:
                        add_dep_helper(mm.ins, gate_dma.ins,
                                       reason="pe warmup gate")

            thrp1, thrn1, gm1 = gamma_chain(acc1, W1C + 1, "g1")

            # --- w1 quant: t1s[s] [128,768] holds -q(w1) for h-slab s ---
            # pass1: a = (w >= g/2); pass2: t = (w <= -g/2) - a  in {-1,0,1}
            t1s = [None] * KH

            def quant_pass2(dst_pool, wsrc, asrc, thrn):
                t = dst_pool.tile([128, 768], FP8,
                                  tag="t1" if dst_pool is t1p else "t2")
                nc.vector.scalar_tensor_tensor(
                    out=t, in0=wsrc, scalar=thrn[:, 0:1], in1=asrc,
                    op0=Alu.is_le, op1=Alu.subtract)
                return t

            for c in range(W1C - 1):
                a = apool.tile([128, 1536], FP8, tag="a")
                if c == 0:
                    # slab-split the first chunk so fc1 can start sooner;
                    # explicit dep pins slab1's pass1 after slab0's pass2
                    prev_p2 = None
                    for sl in (0, 1):
                        s778 = slice(sl * 768, (sl + 1) * 768)
                        p1 = nc.vector.tensor_scalar(
                            out=a[:, s778], in0=wc[0][:, s778],
                            scalar1=thrp1[:, 0:1], scalar2=0.0,
                            op0=Alu.is_ge, op1=Alu.bypass)
                        if prev_p2 is not None:
                            add_dep_helper(p1.ins, prev_p2.ins,
                                           reason="slab0 critical path")
                        t = t1p.tile([128, 768], FP8, tag="t1",
                                     name=f"t1s{sl}")
                        prev_p2 = nc.vector.scalar_tensor_tensor(
                            out=t, in0=wc[0][:, s778],
                            scalar=thrn1[:, 0:1], in1=a[:, s778],
                            op0=Alu.is_le, op1=Alu.subtract)
                        t1s[sl] = t
                else:
                    nc.vector.tensor_scalar(
                        out=a, in0=wc[c], scalar1=thrp1[:, 0:1], scalar2=0.0,
                        op0=Alu.is_ge, op1=Alu.bypass)
                    for sl in (0, 1):
                        s778 = slice(sl * 768, (sl + 1) * 768)
                        t1s[2 * c + sl] = quant_pass2(
                            t1p, wc[c][:, s778], a[:, s778], thrn1)
            for i in (0, 1):
                a = apool.tile([128, 1536], FP8, tag="a")
                nc.vector.tensor_scalar(
                    out=a[:, 0:768], in0=whalf[i], scalar1=thrp1[:, 0:1],
                    scalar2=0.0, op0=Alu.is_ge, op1=Alu.bypass)
                t1s[2 * (W1C - 1) + i] = quant_pass2(
                    t1p, whalf[i], a[:, 0:768], thrn1)

            # --- w2 pass-1 DMAs stream right after x (ring paced by ABS) ---
            acc2 = singles.tile([128, W2C], FP32, tag="acc2")
            w2p1 = []
            for c in range(W2C):
                wf = wf2p.tile([128, 1536], FP32, tag="w2", name=f"w2p1_{c}")
                nc.sync.dma_start(out=wf, in_=wt2[:, c * 1536:(c + 1) * 1536])
                w2p1.append(wf)

            def w2_abs(c):
                absr = absp.tile([128, 1536], BF16, tag="abs",
                                 name=f"absr{c}")
                nc.scalar.activation(absr, w2p1[c], Act.Abs,
                                     accum_out=acc2[:, c:c + 1])

            def w2_quant(c2):
                thrp2, thrn2, gm2 = g2state["thr"]
                wf = wf2p.tile([128, 1536], FP32, tag="w2", name=f"w2p2_{c2}")
                nc.sync.dma_start(out=wf, in_=wt2[:, c2 * 1536:(c2 + 1) * 1536])
                a2 = apool.tile([128, 1536], FP8, tag="a", name=f"a2_{c2}")
                nc.vector.tensor_scalar(
                    out=a2, in0=wf, scalar1=thrp2[:, 0:1], scalar2=0.0,
                    op0=Alu.is_ge, op1=Alu.bypass)
                dc = c2 // 2
                for piece in (0, 1):
                    s778 = slice(piece * 768, (piece + 1) * 768)
                    khb = 2 * (c2 % 2) + piece
                    t2s[dc * 4 + khb] = quant_pass2(
                        t2p, wf[:, s778], a2[:, s778], thrn2)

            # --- fc1; ABS at groups 3..14, gamma2 at 15, quant at 15..23 ---
            htiles = [None] * (KH // 2)
            t2s = [None] * KH
            g2state = {}

            for hc in range(KH):
                ps = psp.tile([128, NT, 512], FP32, tag="ps")
                for kd in range(KD):
                    lhsT = t1s[hc][:, kd * 128:(kd + 1) * 128]
                    for t in range(NT):
                        nc.tensor.matmul(ps[:, t, 0:TN], lhsT,
                                         xb6[kd][:, t * TN:(t + 1) * TN],
                                         start=(kd == 0),
                                         stop=(kd == KD - 1))
                if hc % 2 == 0:
                    htiles[hc // 2] = wstage.tile([128, 2, NT, TN], BF16,
                                                  tag="wc", name=f"h{hc//2}")
                nc.scalar.activation(htiles[hc // 2][:, hc % 2, :, :],
                                     ps[:, :, 0:TN], act_fn,
                                     bias=b1sb[:, hc:hc + 1],
                                     scale=gm1[:, 0:1])
                if 3 <= hc < 3 + W2C:
                    w2_abs(hc - 3)
                if hc >= 15:
                    if hc == 15:
                        g2state["thr"] = gamma_chain(acc2, W2C, "g2")
                    w2_quant(hc - 15)
            for c2 in range(9, 12):
                w2_quant(c2)

            thrp2, thrn2, gm2 = g2state["thr"]

            # --- fc2: t-inner (ldweights reuse); the LAST group goes t-outer
            #     with per-token-tile epilogue+DMA to collapse the tail ---
            for dc in range(KD):
                ps = psp.tile([128, NT, 512], FP32, tag="ps")
                yb = ybp.tile([128, NT, TN], BF16, tag="yb")
                if dc < KD - 1:
                    for kh in range(KH):
                        lhsT = t2s[dc * 4 + kh // 6][:, (kh % 6) * 128:
                                                     (kh % 6 + 1) * 128]
                        for t in range(NT):
                            nc.tensor.matmul(
                                ps[:, t, 0:TN], lhsT,
                                htiles[kh // 2][:, kh % 2, t, :],
                                start=(kh == 0), stop=(kh == KH - 1))
                    nc.vector.tensor_scalar(
                        out=yb, in0=ps[:, :, 0:TN], scalar1=gm2[:, 0:1],
                        scalar2=b2sb[:, dc:dc + 1], op0=Alu.mult, op1=Alu.add)
                    nc.sync.dma_start(
                        out=yt[dc * 128:(dc + 1) * 128, :], in_=yb)
                else:
                    for t in range(NT):
                        for kh in range(KH):
                            lhsT = t2s[dc * 4 + kh // 6][:, (kh % 6) * 128:
                                                         (kh % 6 + 1) * 128]
                            nc.tensor.matmul(
                                ps[:, t, 0:TN], lhsT,
                                htiles[kh // 2][:, kh % 2, t, :],
                                start=(kh == 0), stop=(kh == KH - 1))
                        nc.vector.tensor_scalar(
                            out=yb[:, t, :], in0=ps[:, t, 0:TN],
                            scalar1=gm2[:, 0:1], scalar2=b2sb[:, dc:dc + 1],
                            op0=Alu.mult, op1=Alu.add)
                        nc.sync.dma_start(
                            out=yt[dc * 128:(dc + 1) * 128,
                                   t * TN:(t + 1) * TN],
                            in_=yb[:, t, :])

    nc.compile()
    return nc


_NC = None


def _get_nc():
    global _NC
    if _NC is None:
        _NC = build()
    return _NC


def _host_prep(x, w1, b1, w2, b2):
    x = np.asarray(x, dtype=np.float32)
    w1 = np.asarray(w1, dtype=np.float32)
    b1 = np.asarray(b1, dtype=np.float32)
    w2 = np.asarray(w2, dtype=np.float32)
    b2 = np.asarray(b2, dtype=np.float32)
    x2 = np.ascontiguousarray(x.reshape(TOK, D).T).astype(ml_dtypes.bfloat16)
    # wt1[p, s*768 + kd*128 + i] = w1[s*128+i, kd*128+p]
    wt1 = np.ascontiguousarray(
        w1.reshape(KH, 128, KD, 128).transpose(3, 0, 2, 1).reshape(128, -1))
    # wt2[p, dc*3072 + kh*128 + i] = w2[dc*128+i, kh*128+p]
    wt2 = np.ascontiguousarray(
        w2.reshape(KD, 128, KH, 128).transpose(3, 0, 2, 1).reshape(128, -1))
    b1r = np.ascontiguousarray(b1.reshape(KH, 128).T)
    b2r = np.ascontiguousarray(b2.reshape(KD, 128).T)
    return x2, wt1, wt2, b1r, b2r


def kernel(x, w1, b1, w2, b2, _trace=False, _trace_kwargs=None):
    nc = _get_nc()
    x2, wt1, wt2, b1r, b2r = _host_prep(x, w1, b1, w2, b2)
    in_maps = []
    for c in range(N_CORES):
        in_maps.append({
            "xt": np.ascontiguousarray(x2[:, c * TOK_PER:(c + 1) * TOK_PER]),
            "wt1": wt1, "wt2": wt2, "b1r": b1r, "b2r": b2r,
        })
    out = run_bass_kernel_spmd(nc, in_maps, list(range(N_CORES)),
                               trace=_trace, **(_trace_kwargs or {}))
    res = out.results
    yt = np.concatenate(
        [res[c]["yt"].astype(np.float32) for c in range(N_CORES)], axis=1)
    y = np.ascontiguousarray(yt.T).reshape(B, S, D).astype(np.float32)
    if _trace:
        return y, out
    return y
